# revision 29
# baseline (speedup 1.0000x reference)
"""Trainium2 Bass kernel for CustomHyperSemanticMessagePassing.

Hypergraph multi-head attention message passing, N=4096 nodes, E=4096 edges,
DEG=CARD=8, D=256, H=8 heads.

Sharding: ROUND-parallel. The regular hypergraph is 8 permutation rounds of
512 edges; each core processes one full round for ALL nodes. Within a round
every node belongs to exactly one edge, and laying positions out in permuted
(edge-block) order makes each edge's 8 members contiguous, so per
128-position tile and head:

  - scores are the block-diagonal of ONE PE matmul
        S^T_h = kekT_h^T @ qT_h          (kek = k + ek of the shared edge)
    The off-block entries are suppressed by a +C*sameblock bias folded into
    the SAME matmul as 16 extra contraction rows (sqrt(C)*onehot(block) on
    both operands); after exp, off-block weights are a relative e^-C ~ 1e-13
    of on-block ones, and the uniform e^C factor cancels in the softmax
    normalization.
  - attention weights  wT_h = exp(S^T_h)  (one Activation op, no mask),
  - weighted values + denominator are PE matmuls  ctx_h = wT_h^T @ V_h,
    z_h = wT_h^T @ 1.

Per-round partials (ctx, z) are indirect-scattered to node order (bf16) and
summed across cores (=rounds) with a single ReduceScatter; each core then
finishes its own 512 nodes (normalize, out-proj, relu).

Exact identities used: key bias bk drops (softmax shift invariance); value
bias bv folds into the output bias (softmax weights sum to 1). q bias is
zero in this model. Softmax runs without max-subtraction (scores are O(1)).

Head layout: K/Q projections write 4 chunks x 128 rows; chunk j holds head
2j at rows 0:32, bias rows at 32:48, head 2j+1 at rows 64:96, bias rows at
96:112 (zeros elsewhere), so each head's S matmul reads a 48-deep slice at
base partition 0 or 64 (PE base-partition constraint) that includes its
bias rows.
"""
import numpy as np

import orjson
import concourse.bass as bass
import concourse.tile as tile
import concourse.bass_utils as bass_utils
import concourse.bass2jax as bass2jax
from concourse import mybir
from concourse import library_config

F32 = mybir.dt.float32
BF16 = mybir.dt.bfloat16
I32 = mybir.dt.int32
I16 = mybir.dt.int16

N, E, D, EDGE_DIM = 4096, 4096, 256, 64
H, DH, DEG, CARD = 8, 32, 8, 8
NCORES = 8
NSH = N // NCORES            # nodes owned per core
NPOS = N                     # positions per core (one full round)
NT = NPOS // 128             # 128-position tiles per core
EPC = E // NCORES            # edges per round
SCH = 4                      # tiles per scatter chunk
PC = D + H                   # partial row: 8 heads x (32 ctx + 1 z)
HC = 4                       # head chunks (2 heads + bias rows each)
CBIAS = 30.0                 # on-block score bias (e^-30 off-block leak)


# ---------------------------------------------------------------------------
# walrus workaround: this build accepts only one sync-wait per instruction;
# split extras into injected single-wait NoOps at the BIR-JSON level.
_ORIG_COMPILE = bass_utils.compile_bir_kernel
_ctr = [0]


def _split_multiwaits(bir_json: bytes) -> bytes:
    bir = orjson.loads(bir_json)
    changed = False
    for f in bir.get("functions", []):
        for blk in f.get("blocks", []):
            out = []
            for ins in blk.get("instructions", []):
                si = ins.get("sync_info")
                waits = (si or {}).get("on_wait") or []
                if len(waits) > 1 and ins.get("engine") not in (None, "Unassigned"):
                    changed = True
                    for w in waits[:-1]:
                        _ctr[0] += 1
                        out.append({
                            "debug": ins.get("debug"),
                            "engine": ins["engine"],
                            "ins": [], "outs": [],
                            "name": f"WSPLIT-{_ctr[0]}",
                            "opcode": "NoOp",
                            "sync_info": {"on_wait": [w], "on_update": []},
                        })
                    si["on_wait"] = waits[-1:]
                out.append(ins)
            blk["instructions"] = out
    return orjson.dumps(bir) if changed else bir_json


def _patched_compile(bir_json, tmpdir, neff_name="file.neff"):
    return _ORIG_COMPILE(_split_multiwaits(bytes(bir_json)), tmpdir,
                         neff_name=neff_name)


bass_utils.compile_bir_kernel = _patched_compile
bass2jax.compile_bir_kernel = _patched_compile


# ---------------------------------------------------------------------------
def build_nc():
    nc = bass.Bass(num_devices=NCORES)
    # replicated inputs
    xp_p = nc.declare_dram_parameter("xp", [128, 2, NPOS], BF16,
                                     isOutput=False)
    wblob_p = nc.declare_dram_parameter("wblob", [128, 3200], BF16,
                                        isOutput=False)
    wek_p = nc.declare_dram_parameter("wek_p", [EDGE_DIM, HC, 128], BF16,
                                      isOutput=False)
    bo2_p = nc.declare_dram_parameter("bo2_p", [128, D], F32, isOutput=False)
    # per-core inputs
    scat_p = nc.declare_dram_parameter("scat", [128, NT], I32, isOutput=False)
    ea_p = nc.declare_dram_parameter("ea_own", [EDGE_DIM, EPC], BF16,
                                     isOutput=False)
    # output: this core's own nodes
    out_p = nc.declare_dram_parameter("out", [NSH, D], F32, isOutput=True)

    with tile.TileContext(nc) as tc, \
         tc.tile_pool(name="wp", bufs=1) as wp, \
         tc.tile_pool(name="xg", bufs=2) as xg, \
         tc.tile_pool(name="kp", bufs=6) as kp, \
         tc.tile_pool(name="vp", bufs=4) as vp, \
         tc.tile_pool(name="wtp", bufs=4) as wtp, \
         tc.tile_pool(name="stp", bufs=3) as stp, \
         tc.tile_pool(name="fp", bufs=2) as fp, \
         tc.tile_pool(name="psA", bufs=2, space="PSUM") as psA, \
         tc.tile_pool(name="psV", bufs=1, space="PSUM") as psV, \
         tc.tile_pool(name="psS", bufs=1, space="PSUM") as psS, \
         tc.tile_pool(name="psC", bufs=1, space="PSUM") as psC, \
         tc.tile_pool(name="dram", bufs=1, space="DRAM") as dram:

        # ---- gather indices first (dual-use: gather in, scatter out) ----
        scat_t = wp.tile([128, NT], I32)
        nc.sync.dma_start(out=scat_t[:], in_=scat_p[:])

        # ---- resident weights / constants ----
        wek_t = wp.tile([EDGE_DIM, HC, 128], BF16)
        nc.sync.dma_start(out=wek_t[:], in_=wek_p[:])
        ea_t = wp.tile([EDGE_DIM, EPC], BF16)
        nc.sync.dma_start(out=ea_t[:], in_=ea_p[:])
        wb_t = wp.tile([128, 3200], BF16)
        nc.sync.dma_start(out=wb_t[:], in_=wblob_p[:])
        wk_t = wb_t[:, 0:1024].rearrange("p (a c m) -> p a c m", a=2, c=HC)
        wq_t = wb_t[:, 1024:2048].rearrange("p (a c m) -> p a c m", a=2, c=HC)
        wv_t = wb_t[:, 2048:2560].rearrange("p (a m) -> p a m", a=2)
        owt_t = wb_t[:, 2560:3072].rearrange("p (a m) -> p a m", a=2)
        bpat_t = wb_t[:, 3072:3200].rearrange("p (c b) -> p c b", c=2 * HC)
        bo2_t = wp.tile([128, D], F32)
        nc.sync.dma_start(out=bo2_t[:], in_=bo2_p[:])
        ones_t = wp.tile([128, 1], BF16)
        nc.vector.memset(ones_t[:], 1.0)
        warm_t = wp.tile([128, 1], BF16)
        nc.scalar.activation(out=warm_t[:], in_=ones_t[:],
                             func=mybir.ActivationFunctionType.Exp)

        # ---- ekz: per-edge ekT values (kek chunks) + constant bias rows ----
        # layout [128, 2*HC, EPC]: chunks 0:4 = ekT (+bias rows), 4:8 = bias
        # rows only (q side).  bias rows: sqrt(C)*onehot(e mod 16) at
        # partitions 32:48 and 96:112 of every chunk (bpat pattern).
        ekz = wp.tile([128, 2 * HC, EPC], BF16)
        bpat_bc = bpat_t[:].unsqueeze(2).to_broadcast(
            [128, 2 * HC, EPC // 16, 16])
        for half in range(2):
            pek = psS.tile([128, H, 128], F32, tag="S")
            pek_v = pek[:].rearrange("p (c x) e -> p c (x e)", c=2)
            for i in range(2):
                nc.tensor.matmul(out=pek_v[:, i, :],
                                 lhsT=wek_t[:, 2 * half + i, :], rhs=ea_t[:],
                                 start=True, stop=True)
            nc.vector.tensor_tensor(
                out=ekz[:, 2 * half:2 * half + 2, :].rearrange(
                    "p c (r b) -> p c r b", b=16),
                in0=pek_v[:].rearrange("p c (r b) -> p c r b", b=16),
                in1=bpat_bc[:, 2 * half:2 * half + 2],
                op=mybir.AluOpType.add)
        nc.vector.tensor_copy(
            ekz[:, HC:2 * HC, :].rearrange("p c (r b) -> p c r b", b=16),
            bpat_bc[:, HC:2 * HC])

        # ---- internal DRAM: per-round partials (node order) + RS output ----
        part_t = dram.tile([N, PC], BF16)
        rs_t = dram.tile([NSH, PC], BF16)

        # ---- main loop over 128-position tiles ----
        # 2-stage software pipeline: iteration t emits
        #   proj/TT/va for tile t   (PE -> DVE/ACT)
        #   S/exp      for tile t-1 (PE -> ACT; kekq ready since last iter)
        #   ctx/stage  for tile t-2 (PE -> DVE)
        # so no in-order engine queue ever waits on same-tile producers.
        hist = {}
        xcs = {}

        def fetch_x(g):
            xc = xg.tile([128, 2, 512], BF16, tag="xc")
            nc.sync.dma_start(out=xc[:], in_=xp_p[:, :, bass.ts(g, 512)])
            xcs[g] = xc

        fetch_x(0)
        for t in range(NT + 2):
            if t < NT and t % 4 == 0 and t // 4 + 1 < NT // 4:
                fetch_x(t // 4 + 1)
            if t < NT:
                xt = xcs[t // 4][:, :, bass.ts(t % 4, 128)]

                # in-projections: kekT/qT (d on parts) and v (pos on parts)
                pkq = psA.tile([128, 8, 128], F32, tag="pkqv")
                for ch in range(HC):
                    for kc in range(2):
                        nc.tensor.matmul(out=pkq[:, ch, :],
                                         lhsT=wk_t[:, kc, ch, :],
                                         rhs=xt[:, kc, :],
                                         start=(kc == 0), stop=(kc == 1))
                for ch in range(HC):
                    for kc in range(2):
                        nc.tensor.matmul(out=pkq[:, HC + ch, :],
                                         lhsT=wq_t[:, kc, ch, :],
                                         rhs=xt[:, kc, :],
                                         start=(kc == 0), stop=(kc == 1))
                pv_t = psV.tile([128, D], F32, tag="pv")
                pv = pv_t[:]
                for kc in range(2):
                    nc.tensor.matmul(out=pv, lhsT=xt[:, kc, :],
                                     rhs=wv_t[:, kc, :],
                                     start=(kc == 0), stop=(kc == 1))

                # kekq = proj + (ek | 0) + bias rows, bf16 SBUF (one DVE op)
                kekq = kp.tile([128, 2 * HC, 128], BF16, tag="kekq")
                nc.vector.tensor_tensor(
                    out=kekq[:].rearrange("p c (b j) -> p c b j", b=16),
                    in0=pkq[:, 0:8, :].rearrange("p c (b j) -> p c b j", b=16),
                    in1=ekz[:, :, bass.ts(t, 16)].unsqueeze(3)
                        .to_broadcast([128, 2 * HC, 16, CARD]),
                    op=mybir.AluOpType.add)
                va = vp.tile([128, H, DH], BF16, tag="vaug")
                nc.scalar.copy(out=va[:].rearrange("p h e -> p (h e)"), in_=pv)
                hist[t] = (kekq, va)

            if 1 <= t <= NT:
                kekq1 = hist[t - 1][0]
                # per-head block scores (+C on-block): 48-deep matmul with
                # bias rows
                pS_t = psS.tile([128, H, 128], F32, tag="S")
                pS = pS_t[:]
                for h in range(H):
                    po, ch = 64 * (h % 2), h // 2
                    nc.tensor.matmul(out=pS[:, h, :],
                                     lhsT=kekq1[po:po + 48, ch, :],
                                     rhs=kekq1[po:po + 48, HC + ch, :],
                                     start=True, stop=True)
                wT = wtp.tile([128, H, 128], BF16, tag="wT")
                nc.scalar.activation(out=wT[:], in_=pS[:],
                                     func=mybir.ActivationFunctionType.Exp)
                hist[t - 1] = (None, hist[t - 1][1], wT)

            if t >= 2:
                tp = t - 2
                _, va2, wT2 = hist.pop(tp)
                # per-head [ctx | z] = wT_h^T @ [V_h | 1]  (tile t-2)
                pctx = psC.tile([128, H, DH + 1], F32, tag="ctx")
                for h in range(H):
                    nc.tensor.matmul(out=pctx[:, h, 0:DH],
                                     lhsT=wT2[:, h, :],
                                     rhs=va2[:, h, :], start=True, stop=True)
                    nc.tensor.matmul(out=pctx[:, h, DH:DH + 1],
                                     lhsT=wT2[:, h, :],
                                     rhs=ones_t[:], start=True, stop=True)

                # stage partial row (bf16); scatter this tile to node order
                stage = stp.tile([128, PC], BF16, tag="stage")
                nc.vector.tensor_copy(stage[:],
                                      pctx[:].rearrange("p h e -> p (h e)"))
                nc.gpsimd.indirect_dma_start(
                    out=part_t[:],
                    out_offset=bass.IndirectOffsetOnAxis(
                        ap=scat_t[:, tp:tp + 1], axis=0),
                    in_=stage[:], in_offset=None)

        # ---- combine rounds across cores; keep own node chunk ----
        nc.gpsimd.collective_compute(
            "ReduceScatter", mybir.AluOpType.add,
            replica_groups=[list(range(NCORES))],
            ins=[part_t.opt()], outs=[rs_t.opt()])

        # keep the PE p-state warm while the collective runs (dummy work,
        # no deps on rs_t) so the out-proj matmuls run at full clock
        for wi in range(24):
            pw_t = psA.tile([128, 8, 128], F32, tag="pkqv")
            nc.tensor.matmul(out=pw_t[:, 0, :], lhsT=wb_t[:, 0:128],
                             rhs=wb_t[:, 0:128], start=True, stop=True)

        # ---- finish own nodes: normalize, out-proj, bias, relu ----
        FT = NSH // 128
        ldb = fp.tile([128, FT, H, DH + 1], BF16, tag="ldb")
        nc.sync.dma_start(
            out=ldb[:],
            in_=rs_t[:].rearrange("(f p) (h e) -> p f h e", p=128, h=H))
        oo = fp.tile([128, FT, D], F32, tag="oo")
        for ft in range(FT):
            ld = ldb[:, ft]
            zr = fp.tile([128, H], BF16, tag="zr")
            with nc.allow_low_precision(reason="z reciprocal in bf16"):
                nc.vector.reciprocal(zr[:], ld[:, :, DH])
            cn = fp.tile([128, H, DH], BF16, tag="cn")
            nc.vector.tensor_tensor(
                out=cn[:], in0=ld[:, :, 0:DH],
                in1=zr[:].unsqueeze(2).to_broadcast([128, H, DH]),
                op=mybir.AluOpType.mult)
            cnT = fp.tile([128, 2, 128], BF16, tag="cnT")
            cn_v = cn[:].rearrange("p h e -> p (h e)")
            for dc in range(2):
                nc.sync.dma_start_transpose(cnT[:, dc, :],
                                            cn_v[:, bass.ts(dc, 128)])
            po_t = psA.tile([128, 8, 128], F32, tag="pkqv")
            po = po_t[:].rearrange("p h e -> p (h e)")[:, 0:D]
            for dc in range(2):
                nc.tensor.matmul(out=po, lhsT=cnT[:, dc, :],
                                 rhs=owt_t[:, dc, :],
                                 start=(dc == 0), stop=(dc == 1))
            ob = fp.tile([128, D], F32, tag="ob")
            nc.vector.tensor_tensor(out=ob[:], in0=po, in1=bo2_t[:],
                                    op=mybir.AluOpType.add)
            nc.scalar.activation(out=oo[:, ft, :], in_=ob[:],
                                 func=mybir.ActivationFunctionType.Relu)
        nc.sync.dma_start(
            out=out_p[:].rearrange("(f p) m -> p f m", p=128), in_=oo[:])

    return nc


# ---------------------------------------------------------------------------
def host_prep(x, incidence, edge_attr, W_lin, W_edge,
              in_proj_w, in_proj_b, out_proj_w, out_proj_b):
    import ml_dtypes
    bf = ml_dtypes.bfloat16

    x = np.asarray(x, np.float32)
    inc = np.asarray(incidence, np.float32)
    ea = np.asarray(edge_attr, np.float32)
    W_lin = np.asarray(W_lin, np.float32)
    W_edge = np.asarray(W_edge, np.float32)
    in_proj_w = np.asarray(in_proj_w, np.float32)
    in_proj_b = np.asarray(in_proj_b, np.float32)
    out_proj_w = np.asarray(out_proj_w, np.float32)
    out_proj_b = np.asarray(out_proj_b, np.float32)

    # members per edge; rounds are contiguous blocks of EPC edges
    noe = np.nonzero(inc)[1].reshape(E, CARD).astype(np.int64)

    Wq, Wk, Wv = in_proj_w[0:D], in_proj_w[D:2 * D], in_proj_w[2 * D:3 * D]
    bq, bv = in_proj_b[0:D], in_proj_b[2 * D:3 * D]
    assert not np.any(bq), "nonzero q bias not supported by this kernel"
    scale = 1.0 / np.sqrt(np.float32(DH))

    wkc = W_lin @ Wk.T                     # [D, D]
    wvc = W_lin @ Wv.T
    wqc = W_lin @ Wq.T * scale
    wek = W_edge @ Wk.T                    # [EDGE_DIM, D]
    owt = out_proj_w.T.copy()              # [D, D]
    bo2 = out_proj_b + bv @ out_proj_w.T   # bv folds through (sum w = 1)

    def pack(w):  # [D, D] -> [128, 2, D] with [k, kc, :] = w[kc*128+k, :]
        return np.ascontiguousarray(
            w.reshape(2, 128, D).transpose(1, 0, 2)).astype(bf)

    def pad_heads(w):
        # [in, D] -> [in, HC, 128]: chunk j = head 2j cols at 0:32,
        # head 2j+1 cols at 64:96, zeros elsewhere (bias rows live there)
        k = w.shape[0]
        out = np.zeros((k, HC, 128), np.float32)
        for j in range(HC):
            out[:, j, 0:DH] = w[:, (2 * j) * DH:(2 * j + 1) * DH]
            out[:, j, 64:64 + DH] = w[:, (2 * j + 1) * DH:(2 * j + 2) * DH]
        return out

    def pack_heads(w):  # [D, D] -> [128, 2, HC, 128] (k-chunked + head-pad)
        p = pad_heads(w)  # [D, HC, 128]
        return np.ascontiguousarray(
            p.reshape(2, 128, HC, 128).transpose(1, 0, 2, 3)).astype(bf)

    # bias-row pattern: sqrt(C)*onehot(block) at partitions 32:48 and 96:112
    # of every chunk (both kek- and q-side chunks)
    bpat = np.zeros((128, 2 * HC, 16), np.float32)
    rt = np.sqrt(np.float32(CBIAS))
    for r in range(16):
        bpat[32 + r, :, r] = rt
        bpat[96 + r, :, r] = rt

    wblob = np.concatenate([
        pack_heads(wkc).reshape(128, 1024),
        pack_heads(wqc).reshape(128, 1024),
        pack(wvc).reshape(128, 512),
        pack(owt).reshape(128, 512),
        bpat.astype(bf).reshape(128, 128),
    ], axis=1)
    rep = dict(
        wblob=np.ascontiguousarray(wblob),
        wek_p=pad_heads(wek).astype(bf),
        bo2_p=np.broadcast_to(bo2, (128, D)).copy(),
    )
    per_core = []
    for c in range(NCORES):
        perm = noe[c * EPC:(c + 1) * EPC].reshape(-1)     # [NPOS]
        scat = perm.reshape(NT, 128).T.astype(np.int32)   # [128, NT]
        m = dict(rep)
        m["scat"] = np.ascontiguousarray(scat)
        # xT in this core's permuted position order: [k, kc, i]
        m["xp"] = np.ascontiguousarray(
            x.T[:, perm].reshape(2, 128, NPOS).transpose(1, 0, 2)).astype(bf)
        m["ea_own"] = np.ascontiguousarray(
            ea[c * EPC:(c + 1) * EPC].T).astype(bf)
        per_core.append(m)
    return per_core


_CACHE = {}


def kernel(x, incidence, edge_attr, W_lin, W_edge,
           in_proj_w, in_proj_b, out_proj_w, out_proj_b, deg, card):
    assert int(deg) == DEG and int(card) == CARD
    in_maps = host_prep(x, incidence, edge_attr, W_lin, W_edge,
                        in_proj_w, in_proj_b, out_proj_w, out_proj_b)
    if "nc" not in _CACHE:
        _CACHE["nc"] = build_nc()
    from concourse.bass_utils import run_bass_kernel_spmd
    res = run_bass_kernel_spmd(_CACHE["nc"], in_maps, list(range(NCORES)))
    return np.concatenate([res.results[c]["out"] for c in range(NCORES)],
                          axis=0)
